# revision 1
# baseline (speedup 1.0000x reference)
"""TRN2 Bass kernel for nn_ONOBlock (linear attention + MLPs + covariance whitening).

Sharding: data-parallel over batch, 1 batch element per core (B=8, n_cores=8).
Two launches with a host boundary for the [64,64] covariance all-reduce + Cholesky:
  fx_out = X_ @ (L^-T diag(softplus(mu)) L^-1) @ (X_^T fx)
so the per-token whitening matmul disappears and only cov crosses cores.

All heavy matmuls run as float32r (round-to-nearest-11-bit-mantissa, 1 cy/row,
measured 1.5e-4 rel err). LN gains fold into the following weights on the host;
zero biases are skipped at build time (rank-1 ones-matmul fallback if nonzero).
"""
import contextlib
import numpy as np

import bass_rust as _bass_rust
import concourse.bass as bass
import concourse.bacc as bacc
import concourse.tile as tile
from concourse import mybir
from concourse.hw_specs import get_activation_tables
from concourse.bass_utils import run_bass_kernel_spmd
from concourse.masks import make_identity

class _Bacc(bacc.Bacc):
    """Bacc with act-table selection steered to the combined ln+exp set.

    The stock pass resolves Ln->'natural_log' and Exp->'exp_and_others',
    reloading the ACT table between them (~1.3us each, every chunk).
    Masking those two sets forces both onto 'natural_log_exp_and_others'."""

    def insert_act_table_loads(self):
        has_activation = any(
            isinstance(i, mybir.InstActivation)
            for b in self.main_func.blocks
            for i in b.instructions
        )
        if not has_activation:
            return
        tabs = [
            (nm, (set() if nm in ("natural_log", "exp_and_others") else fs))
            for nm, fs in get_activation_tables(self.m.arch).items()
        ]
        _bass_rust.insert_act_table_loads(self, tabs)


F32 = mybir.dt.float32
F32R = mybir.dt.float32r
AF = mybir.ActivationFunctionType
ALU = mybir.AluOpType
AX = mybir.AxisListType

B, N, D, H, PSI = 8, 7225, 256, 8, 64
DH = D // H
DF = 4 * D
EPS = 1e-5
NP_ = 7232            # padded sequence: 56*128 + 64
NCH1 = 57             # pass-1 chunks (56 of 128 + 1 of 64)
NCH2 = 15             # pass-2 chunks (14 of 512 + 1 of 64)
CORES = list(range(8))


def _bcast(ap, parts):
    """Free-dim broadcast helper: [p, g] -> [p, g, parts] with 0-stride."""
    return bass.AP(tensor=ap.tensor, offset=ap.offset,
                   ap=[ap.ap[0], ap.ap[1], [0, parts]])


def _ln_stats(nc, pool, x_ap, w, mv_slot):
    """bn stats into mv_slot [w, 2] = (mean, var)."""
    stats = pool.tile([128, 6], F32, tag="ln_stats")
    nc.vector.bn_stats(out=stats[0:w], in_=x_ap)
    nc.vector.bn_aggr(out=mv_slot, in_=stats[0:w])


def _ln_rstd(nc, rstd_out, var_ap, eps_t):
    """rstd = exp(-0.5*ln(var+eps)); Ln and Exp share ACT func set 6 (no table switch)."""
    nc.scalar.activation(rstd_out, var_ap, AF.Ln, bias=eps_t)
    nc.scalar.activation(rstd_out, rstd_out, AF.Exp, scale=-0.5)


I32 = mybir.dt.int32


def _dve_rsqrt(nc, pool, var_ap, w, n, rstd_out, eps, magic):
    """rstd_out[0:w, 0:n] = 1/sqrt(var_ap + eps) entirely on DVE.

    Quake bit-trick init + 2 Newton steps; ~1e-5 rel err. Keeps the ACT
    engine free of Sqrt/Ln (which share no table set with Gelu)."""
    v4 = pool.tile([128, 4], F32, tag="rs_v")
    nc.vector.tensor_scalar(out=v4[0:w, 0:n], in0=var_ap, scalar1=float(eps),
                            scalar2=None, op0=ALU.add)
    sh = pool.tile([128, 4], I32, tag="rs_sh")
    nc.vector.tensor_scalar(out=sh[0:w, 0:n], in0=v4[0:w, 0:n].bitcast(I32),
                            scalar1=1, scalar2=None, op0=ALU.logical_shift_right)
    y = rstd_out
    nc.vector.tensor_tensor(out=y[0:w, 0:n].bitcast(I32), in0=magic[0:w, 0:n],
                            in1=sh[0:w, 0:n], op=ALU.subtract)
    t = pool.tile([128, 4], F32, tag="rs_t")
    for _ in range(2):
        nc.vector.tensor_tensor(out=t[0:w, 0:n], in0=y[0:w, 0:n], in1=y[0:w, 0:n], op=ALU.mult)
        nc.vector.tensor_tensor(out=t[0:w, 0:n], in0=t[0:w, 0:n], in1=v4[0:w, 0:n], op=ALU.mult)
        nc.vector.tensor_scalar(out=t[0:w, 0:n], in0=t[0:w, 0:n], scalar1=-0.5,
                                scalar2=1.5, op0=ALU.mult, op1=ALU.add)
        nc.vector.tensor_tensor(out=y[0:w, 0:n], in0=y[0:w, 0:n], in1=t[0:w, 0:n], op=ALU.mult)


def _ln_apply(nc, h_out, x_ap, mean_ap, rstd_ap, w):
    nc.vector.tensor_scalar(out=h_out[0:w], in0=x_ap, scalar1=mean_ap,
                            scalar2=rstd_ap, op0=ALU.subtract, op1=ALU.mult)


def _ln_ops(nc, pool, x_ap, w, h_out, eps_t):
    """Single-sub LayerNorm (gain/bias folded into weights)."""
    mv = pool.tile([128, 2], F32, tag="ln_mv")
    _ln_stats(nc, pool, x_ap, w, mv[0:w])
    rstd = pool.tile([128, 1], F32, tag="ln_rstd")
    _ln_rstd(nc, rstd[0:w], mv[0:w, 1:2], eps_t[0:w])
    _ln_apply(nc, h_out, x_ap, mv[0:w, 0:1], rstd[0:w], w)


def _transpose_pair(nc, ptr_pool, ident_m, src, w, dst_ap, copy_eng):
    """PE-transpose src[0:w, 0:128] and src[0:w, 128:256] into one psum tile,
    then a single copy to dst_ap ([128, 2, w] view). ident_m matches src dtype."""
    dt_ = src.dtype
    pt = ptr_pool.tile([128, 256], dt_, tag="tr", name="pt")
    for dc in range(2):
        nc.tensor.matmul(pt[:, dc * w:(dc + 1) * w], src[0:w, dc * 128:(dc + 1) * 128],
                         ident_m[0:w, 0:w], is_transpose=True,
                         skip_group_check=(dc == 1))
    copy_eng(dst_ap, pt[:, 0:2 * w].rearrange("p (c w) -> p c w", c=2))


def build_launch1(flags):
    nc = _Bacc(None)
    # ---- I/O ----
    x_d = nc.dram_tensor("x", [NP_, D], F32, kind="ExternalInput")
    fx_d = nc.dram_tensor("fx", [NP_, D], F32R, kind="ExternalInput")
    wqkv_d = nc.dram_tensor("wqkv", [D, 3 * D], F32R, kind="ExternalInput")
    wo_d = nc.dram_tensor("wo", [D, D], F32R, kind="ExternalInput")
    w1_d = nc.dram_tensor("w1", [D, DF], F32R, kind="ExternalInput")
    w2_d = nc.dram_tensor("w2", [DF, D], F32R, kind="ExternalInput")
    p1_d = nc.dram_tensor("p1", [D, D], F32R, kind="ExternalInput")
    p2_d = nc.dram_tensor("p2", [D, PSI], F32R, kind="ExternalInput")
    cmask_d = nc.dram_tensor("cmask", [D, D], F32, kind="ExternalInput")
    ib1_d = nc.dram_tensor("ib1", [DF], F32, kind="ExternalInput")
    ip1_d = nc.dram_tensor("ip1", [D], F32, kind="ExternalInput")
    ipb2_d = nc.dram_tensor("ipb2", [PSI], F32, kind="ExternalInput")
    if flags["bqkv"]:
        bqkv_d = nc.dram_tensor("bqkv", [1, 3 * D], F32R, kind="ExternalInput")
    if flags["bo"]:
        bo_d = nc.dram_tensor("bo", [1, D], F32R, kind="ExternalInput")
    if flags["b2"]:
        b2_d = nc.dram_tensor("b2", [1, D], F32R, kind="ExternalInput")

    x2o_d = nc.dram_tensor("x2o", [NP_, D], F32, kind="ExternalOutput")
    xt_d = nc.dram_tensor("xt", [PSI, NP_], F32, kind="ExternalOutput")
    cov_d = nc.dram_tensor("cov", [PSI, PSI], F32, kind="ExternalOutput")
    c2p_d = nc.dram_tensor("c2p", [PSI, D], F32, kind="ExternalOutput")

    with tile.TileContext(nc) as tc, contextlib.ExitStack() as top:
        wp = top.enter_context(tc.tile_pool(name="wp", bufs=1))
        # ---- resident weights/constants ----
        wqkv = wp.tile([128, 2, 3 * D], F32R)
        nc.sync.dma_start(out=wqkv, in_=wqkv_d.rearrange("(c p) e -> p c e", p=128))
        wo = wp.tile([128, 2, D], F32R)
        nc.sync.dma_start(out=wo, in_=wo_d.rearrange("(c p) e -> p c e", p=128))
        w1 = wp.tile([128, 2, DF], F32R)
        nc.sync.dma_start(out=w1, in_=w1_d.rearrange("(c p) e -> p c e", p=128))
        w2 = wp.tile([128, 8, D], F32R)
        nc.sync.dma_start(out=w2, in_=w2_d.rearrange("(c p) e -> p c e", p=128))
        p1 = wp.tile([128, 2, D], F32R)
        nc.sync.dma_start(out=p1, in_=p1_d.rearrange("(c p) e -> p c e", p=128))
        p2 = wp.tile([128, 2, PSI], F32R)
        nc.sync.dma_start(out=p2, in_=p2_d.rearrange("(c p) e -> p c e", p=128))
        cmask = wp.tile([128, 2, D], F32)
        nc.sync.dma_start(out=cmask, in_=cmask_d.rearrange("(c p) e -> p c e", p=128))
        ib1 = wp.tile([128, 8], F32)
        nc.sync.dma_start(out=ib1, in_=ib1_d.rearrange("(a p) -> p a", p=128))
        ip1 = wp.tile([128, 2], F32)
        nc.sync.dma_start(out=ip1, in_=ip1_d.rearrange("(a p) -> p a", p=128))
        ipb2 = wp.tile([64, 1], F32)
        nc.sync.dma_start(out=ipb2, in_=ipb2_d.rearrange("(p a) -> p a", a=1))
        if flags["bqkv"]:
            bqkv = wp.tile([1, 3 * D], F32R)
            nc.sync.dma_start(out=bqkv, in_=bqkv_d[:])
        if flags["bo"]:
            bo = wp.tile([1, D], F32R)
            nc.sync.dma_start(out=bo, in_=bo_d[:])
        if flags["b2"]:
            b2 = wp.tile([1, D], F32R)
            nc.sync.dma_start(out=b2, in_=b2_d[:])

        eps_t = wp.tile([128, 1], F32)
        nc.vector.memset(eps_t, EPS)
        magic = wp.tile([128, 4], I32)
        nc.vector.memset(magic, 0x5F3759DF)
        ident = wp.tile([128, 128], F32)
        make_identity(nc, ident)
        ident_r = wp.tile([128, 128], F32R)
        nc.vector.tensor_copy(ident_r, ident)
        ones_f = wp.tile([128, 16], F32)
        nc.vector.memset(ones_f, 1.0)
        ones_col = wp.tile([128, 1], F32R)
        nc.vector.tensor_copy(ones_col, ones_f[:, 0:1])
        onesc_r = wp.tile([128, 2], F32R)
        nc.vector.tensor_copy(onesc_r, ones_f[:, 0:2])
        zero_f = wp.tile([128, 16], F32)
        nc.vector.memset(zero_f, 0.0)

        qT = wp.tile([128, 2, NP_], F32R)      # q softmax'd, transposed, resident
        C_sb = wp.tile([128, 2, D], F32R)      # masked/scaled context matrix
        CW_sb = wp.tile([128, 2, D], F32R)     # C @ Wo

        # ================= PASS 1 =================
        with contextlib.ExitStack() as s1:
            sb = s1.enter_context(tc.tile_pool(name="p1sb", bufs=4))
            pctx = s1.enter_context(tc.tile_pool(name="pctx", bufs=1, space="PSUM"))
            pqk = s1.enter_context(tc.tile_pool(name="pqk", bufs=2, space="PSUM"))
            pv = s1.enter_context(tc.tile_pool(name="pv", bufs=1, space="PSUM"))
            ptr = s1.enter_context(tc.tile_pool(name="ptr", bufs=3, space="PSUM"))

            ctx_ps = [pctx.tile([128, 264], F32, tag=f"ctx{dc}", name=f"ctx_ps{dc}")
                      for dc in range(2)]

            def p1dim(c):
                t0 = c * 128
                w = 128 if c < NCH1 - 1 else 64
                return t0, w

            def p1_front(c):
                t0, w = p1dim(c)
                x_sb = sb.tile([128, D], F32, tag="x_in", name="x_sb")
                nc.sync.dma_start(out=x_sb[0:w], in_=x_d[t0:t0 + w, :])
                h0 = sb.tile([128, D], F32R, tag="h0", name="h0")
                _ln_ops(nc, sb, x_sb[0:w], w, h0, eps_t)
                h0T = sb.tile([128, 2, 128], F32R, tag="h0T", name="h0T")
                _transpose_pair(nc, ptr, ident_r, h0, w, h0T[:, :, 0:w],
                                lambda d_, s_: nc.vector.tensor_copy(d_, s_))
                return h0T

            def p1_qkv(c, h0T):
                t0, w = p1dim(c)
                ps_qk = pqk.tile([128, 2 * D], F32, tag="qk", name="ps_qk")
                ps_v = pv.tile([128, D], F32, tag="v", name="ps_v")
                for i in range(2):
                    for dc in range(2):
                        nc.tensor.matmul(ps_qk[0:w, i * D:(i + 1) * D], h0T[:, dc, 0:w],
                                         wqkv[:, dc, i * D:(i + 1) * D],
                                         start=(dc == 0 and i == 0),
                                         stop=(dc == 1 and not flags["bqkv"]),
                                         skip_group_check=(i == 1))
                    if flags["bqkv"]:
                        nc.tensor.matmul(ps_qk[0:w, i * D:(i + 1) * D],
                                         ones_col[0:1, 0:1].broadcast_to([1, w]),
                                         bqkv[:, i * D:(i + 1) * D], start=False, stop=True,
                                         skip_group_check=True)
                for dc in range(2):
                    nc.tensor.matmul(ps_v[0:w], h0T[:, dc, 0:w],
                                     wqkv[:, dc, 2 * D:3 * D],
                                     start=(dc == 0), stop=(dc == 1 and not flags["bqkv"]))
                if flags["bqkv"]:
                    nc.tensor.matmul(ps_v[0:w], ones_col[0:1, 0:1].broadcast_to([1, w]),
                                     bqkv[:, 2 * D:3 * D], start=False, stop=True)
                return ps_qk, ps_v

            def p1_back(c, ps_qk, ps_v):
                t0, w = p1dim(c)
                eqk = sb.tile([128, 2 * D], F32R, tag="eqk", name="eqk")
                nc.scalar.activation(eqk[0:w], ps_qk[0:w], AF.Exp)
                eq = eqk[:, 0:D]
                ek = eqk[:, D:2 * D]
                qs = sb.tile([128, 8], F32, tag="qs", name="qs")
                nc.vector.reduce_sum(out=qs[0:w], in_=eq[0:w].rearrange("p (g s) -> p g s", g=8), axis=AX.X)
                nc.vector.reciprocal(qs[0:w], qs[0:w])
                q_sm = sb.tile([128, D], F32R, tag="q_sm", name="q_sm")
                nc.gpsimd.tensor_tensor(out=q_sm[0:w].rearrange("p (g s) -> p g s", g=8),
                                        in0=eq[0:w].rearrange("p (g s) -> p g s", g=8),
                                        in1=_bcast(qs[0:w], 32), op=ALU.mult)
                _transpose_pair(nc, ptr, ident_r, q_sm, w, qT[:, :, t0:t0 + w],
                                lambda d_, s_: nc.scalar.activation(d_, s_, AF.Copy))

                v_sb = sb.tile([128, D], F32R, tag="v_sb", name="v_sb")
                nc.scalar.activation(v_sb[0:w], ps_v[0:w], AF.Copy)
                kv = w if c < NCH1 - 1 else N - t0
                for dc in range(2):
                    nc.tensor.matmul(ctx_ps[dc][:, 0:D], ek[0:kv, dc * 128:(dc + 1) * 128],
                                     v_sb[0:kv], start=(c == 0), stop=(c == NCH1 - 1))
                    nc.tensor.matmul(ctx_ps[dc][:, 256:258], ek[0:kv, dc * 128:(dc + 1) * 128],
                                     onesc_r[0:kv], start=False, stop=(c == NCH1 - 1),
                                     skip_group_check=True)

            h0T_c = p1_front(0)
            for c in range(NCH1):
                qkv = p1_qkv(c, h0T_c)
                h0T_c = p1_front(c + 1) if c + 1 < NCH1 else None
                p1_back(c, *qkv)

            for dc in range(2):
                nc.vector.tensor_copy(qT[:, dc, N:NP_], zero_f[:, 0:NP_ - N])

            # ---- build C = blockdiag_mask * DH^-0.5 * diag(1/Z) @ ctx ----
            for dc in range(2):
                zr = sb.tile([128, 1], F32, tag="zr")
                nc.vector.reciprocal(zr, ctx_ps[dc][:, 256:257])
                ct = sb.tile([128, D], F32, tag="ct")
                nc.vector.tensor_scalar(out=ct, in0=ctx_ps[dc][:, 0:D], scalar1=zr,
                                        scalar2=None, op0=ALU.mult)
                nc.vector.tensor_tensor(out=C_sb[:, dc, :], in0=ct, in1=cmask[:, dc, :], op=ALU.mult)
            # CT = C^T, then CW = C @ Wo  (x1 = q_sm @ C @ Wo, associativity)
            CT_sb = wp.tile([128, 2, D], F32R)
            for dc in range(2):
                _transpose_pair(nc, ptr, ident_r, C_sb[:, dc, :], 128, CT_sb[:, :, dc * 128:(dc + 1) * 128].rearrange("p c w -> p c w"),
                                lambda d_, s_: nc.vector.tensor_copy(d_, s_))
            for m in range(2):
                cwps = pqk.tile([128, 2 * D], F32, tag="qk", name="cwps")
                for ec in range(2):
                    nc.tensor.matmul(cwps[:, 0:D], CT_sb[:, ec, m * 128:(m + 1) * 128],
                                     wo[:, ec, :], start=(ec == 0), stop=(ec == 1))
                nc.vector.tensor_copy(CW_sb[:, m, :], cwps[:, 0:D])

        # ================= PASS 2 =================
        with contextlib.ExitStack() as s2:
            sb = s2.enter_context(tc.tile_pool(name="p2sb", bufs=3))
            sb3 = s2.enter_context(tc.tile_pool(name="p2sb3", bufs=4))
            pcc = s2.enter_context(tc.tile_pool(name="pcc", bufs=1, space="PSUM"))
            pbig = s2.enter_context(tc.tile_pool(name="pbig", bufs=3, space="PSUM"))
            px2 = s2.enter_context(tc.tile_pool(name="px2", bufs=1, space="PSUM"))
            ptr = s2.enter_context(tc.tile_pool(name="ptr2", bufs=2, space="PSUM"))

            cc_ps = pcc.tile([64, 320], F32)

            def chdim(C):
                T0 = C * 512
                T = 512 if C < NCH2 - 1 else 64
                nsub = T // 128 if C < NCH2 - 1 else 1
                sw = 128 if C < NCH2 - 1 else 64
                return T0, T, nsub, sw

            def front(C):
                """attention apply + residual + LN2 -> h2T for chunk C."""
                T0, T, nsub, sw = chdim(C)
                x1_sb = sb.tile([128, 4, D], F32, tag="x1", name="x1_sb")
                h2T = sb.tile([128, 2, 512], F32R, tag="h2T", name="h2T")
                mv4 = sb.tile([128, 4, 2], F32, tag="mv4", name="mv4")
                rstd4 = sb.tile([128, 4], F32, tag="rstd4", name="rstd4")
                for s in range(nsub):
                    t0 = T0 + s * 128
                    xps = pbig.tile([128, 512], F32, tag="big", name="xps")
                    for dc in range(2):
                        nc.tensor.matmul(xps[0:sw, 0:D], qT[:, dc, t0:t0 + sw],
                                         CW_sb[:, dc, :],
                                         start=(dc == 0), stop=(dc == 1 and not flags["bo"]))
                    if flags["bo"]:
                        nc.tensor.matmul(xps[0:sw, 0:D], ones_col[0:1, 0:1].broadcast_to([1, sw]),
                                         bo[:], start=False, stop=True)
                    x_in = sb3.tile([128, D], F32, tag="x_in2", name="x_in")
                    nc.sync.dma_start(out=x_in[0:sw], in_=x_d[t0:t0 + sw, :])
                    nc.vector.tensor_tensor(out=x1_sb[0:sw, s, :], in0=xps[0:sw, 0:D],
                                            in1=x_in[0:sw], op=ALU.add)
                    _ln_stats(nc, sb3, x1_sb[0:sw, s, :], sw, mv4[0:sw, s, :])
                    pass
                _dve_rsqrt(nc, sb3, mv4[0:sw, 0:nsub, 1:2], sw, nsub, rstd4, EPS, magic)
                for s in range(nsub):
                    h2 = sb3.tile([128, D], F32R, tag="h2", name="h2")
                    _ln_apply(nc, h2, x1_sb[0:sw, s, :], mv4[0:sw, s, 0:1],
                              rstd4[0:sw, s:s + 1], sw)
                    _transpose_pair(nc, ptr, ident_r, h2, sw,
                                    h2T[:, :, s * 128:s * 128 + sw],
                                    lambda d_, s_: nc.vector.tensor_copy(d_, s_))
                return x1_sb, h2T

            def back_mlp(C, st):
                """u/gelu/x2-accumulate for chunk C."""
                T0, T, nsub, sw = chdim(C)
                x1_sb, h2T = st
                x2acc = px2.tile([128, 4, D], F32, tag="x2acc", name="x2acc")
                for fs in range(8):
                    ups = pbig.tile([128, 512], F32, tag="big", name="ups")
                    for dc in range(2):
                        nc.tensor.matmul(ups[:, 0:T], w1[:, dc, fs * 128:(fs + 1) * 128],
                                         h2T[:, dc, 0:T], start=(dc == 0), stop=(dc == 1))
                    uT = sb3.tile([128, 512], F32R, tag="uT", name="uT")
                    nc.scalar.activation(uT[:, 0:T], ups[:, 0:T], AF.Gelu,
                                         bias=ib1[:, fs:fs + 1])
                    for s in range(nsub):
                        nc.tensor.matmul(x2acc[0:sw, s, :], uT[:, s * 128:s * 128 + sw],
                                         w2[:, fs, :],
                                         start=(fs == 0 and s % 2 == 0),
                                         stop=(fs == 7 and not flags["b2"]),
                                         skip_group_check=(fs > 0 or s % 2 == 1))
                if flags["b2"]:
                    for s in range(nsub):
                        nc.tensor.matmul(x2acc[0:sw, s, :], ones_col[0:1, 0:1].broadcast_to([1, sw]),
                                         b2[:], start=False, stop=True, skip_group_check=True)
                return x2acc

            def back_tail(C, st, x2acc):
                T0, T, nsub, sw = chdim(C)
                x1_sb, h2T = st
                x2T = sb.tile([128, 2, 512], F32R, tag="x2T", name="x2T")
                for s in range(nsub):
                    t0 = T0 + s * 128
                    x2_sb = sb3.tile([128, D], F32, tag="x2_sb", name="x2_sb")
                    nc.vector.tensor_tensor(out=x2_sb[0:sw], in0=x2acc[0:sw, s, :],
                                            in1=x1_sb[0:sw, s, :], op=ALU.add)
                    nc.sync.dma_start(out=x2o_d[t0:t0 + sw, :], in_=x2_sb[0:sw])
                    _transpose_pair(nc, ptr, ident, x2_sb, sw,
                                    x2T[:, :, s * 128:s * 128 + sw],
                                    lambda d_, s_: nc.scalar.activation(d_, s_, AF.Copy))

                pT = sb.tile([128, 2, 512], F32R, tag="pT", name="pT")
                for pc in range(2):
                    pps = pbig.tile([128, 512], F32, tag="big", name="pps")
                    for dc in range(2):
                        nc.tensor.matmul(pps[:, 0:T], p1[:, dc, pc * 128:(pc + 1) * 128],
                                         x2T[:, dc, 0:T], start=(dc == 0), stop=(dc == 1))
                    nc.scalar.activation(pT[:, pc, 0:T], pps[:, 0:T], AF.Gelu,
                                         bias=ip1[:, pc:pc + 1])
                xtps = pbig.tile([128, 512], F32, tag="big", name="xtps")
                for pc in range(2):
                    nc.tensor.matmul(xtps[0:64, 0:T], p2[:, pc, :], pT[:, pc, 0:T],
                                     start=(pc == 0), stop=(pc == 1))
                xT_sb = sb.tile([64, 512], F32R, tag="xT_sb", name="xT_sb")
                nc.scalar.activation(xT_sb[:, 0:T], xtps[0:64, 0:T], AF.Identity,
                                     bias=ipb2[:, 0:1])
                nc.sync.dma_start(out=xt_d[:, T0:T0 + T], in_=xT_sb[:, 0:T].bitcast(F32))

                for s in range(nsub):
                    t0 = T0 + s * 128
                    vv = min(sw, N - t0)
                    xc = sb3.tile([128, 320], F32R, tag="xc", name="xc")
                    xtr = ptr.tile([128, 128], F32R, tag="tr", name="xtr")
                    nc.tensor.transpose(xtr[0:sw, 0:64], xT_sb[:, s * 128:s * 128 + sw],
                                        ident_r[0:64, 0:64])
                    if vv < sw and flags.get("anybias"):
                        nc.vector.tensor_copy(xc[0:sw, :],
                                              _bcast(zero_f[0:sw, 0:1], 320).rearrange("p a b -> p (a b)"))
                        nc.vector.tensor_copy(xc[0:vv, 0:64], xtr[0:vv, 0:64])
                    else:
                        nc.vector.tensor_copy(xc[0:sw, 0:64], xtr[0:sw, 0:64])
                    nc.sync.dma_start(out=xc[0:sw, 64:320], in_=fx_d[t0:t0 + sw, :])
                    nc.tensor.matmul(cc_ps, xc[0:sw, 0:64], xc[0:sw, :],
                                     start=(C == 0 and s == 0),
                                     stop=(C == NCH2 - 1 and s == nsub - 1))

            # software pipeline: front(C+1) emitted between MLP(C) and tail(C)
            st = front(0)
            for C in range(NCH2):
                x2acc = back_mlp(C, st)
                back_tail(C, st, x2acc)
                st = front(C + 1) if C + 1 < NCH2 else None

            cc_sb = sb.tile([64, 320], F32, tag="cc_sb")
            nc.vector.tensor_copy(cc_sb, cc_ps)
            nc.sync.dma_start(out=cov_d[:], in_=cc_sb[:, 0:64])
            nc.sync.dma_start(out=c2p_d[:], in_=cc_sb[:, 64:320])

    nc.finalize()
    return nc


def build_launch2(flags):
    nc = _Bacc(None)
    xt_d = nc.dram_tensor("xt", [PSI, NP_], F32R, kind="ExternalInput")
    c2pp_d = nc.dram_tensor("c2pp", [PSI, D], F32R, kind="ExternalInput")
    m1_d = nc.dram_tensor("m1", [D, DF], F32R, kind="ExternalInput")
    m2_d = nc.dram_tensor("m2", [DF, D], F32R, kind="ExternalInput")
    ib2_d = nc.dram_tensor("ib2", [DF], F32, kind="ExternalInput")
    if flags["mb2"]:
        mb2_d = nc.dram_tensor("mb2", [1, D], F32R, kind="ExternalInput")
    fxo_d = nc.dram_tensor("fxo", [NP_, D], F32, kind="ExternalOutput")

    with tile.TileContext(nc) as tc, contextlib.ExitStack() as top:
        wp = top.enter_context(tc.tile_pool(name="wp", bufs=1))
        xt_all = wp.tile([64, NP_], F32R)
        nc.sync.dma_start(out=xt_all, in_=xt_d[:])
        c2pp = wp.tile([64, D], F32R)
        nc.sync.dma_start(out=c2pp, in_=c2pp_d[:])
        m1 = wp.tile([128, 2, DF], F32R)
        nc.sync.dma_start(out=m1, in_=m1_d.rearrange("(c p) e -> p c e", p=128))
        m2 = wp.tile([128, 8, D], F32R)
        nc.sync.dma_start(out=m2, in_=m2_d.rearrange("(c p) e -> p c e", p=128))
        ib2 = wp.tile([128, 8], F32)
        nc.sync.dma_start(out=ib2, in_=ib2_d.rearrange("(a p) -> p a", p=128))
        if flags["mb2"]:
            mb2 = wp.tile([1, D], F32R)
            nc.sync.dma_start(out=mb2, in_=mb2_d[:])
            ones_f = wp.tile([128, 1], F32)
            nc.vector.memset(ones_f, 1.0)
            ones_col = wp.tile([128, 1], F32R)
            nc.vector.tensor_copy(ones_col, ones_f)
        eps_t = wp.tile([128, 1], F32)
        nc.vector.memset(eps_t, EPS)
        magic = wp.tile([128, 4], I32)
        nc.vector.memset(magic, 0x5F3759DF)
        ident = wp.tile([128, 128], F32)
        make_identity(nc, ident)
        ident_r = wp.tile([128, 128], F32R)
        nc.vector.tensor_copy(ident_r, ident)

        with contextlib.ExitStack() as s1:
            sb = s1.enter_context(tc.tile_pool(name="sb", bufs=3))
            sb3 = s1.enter_context(tc.tile_pool(name="sb3", bufs=4))
            pbig = s1.enter_context(tc.tile_pool(name="pbig", bufs=2, space="PSUM"))
            pmid = s1.enter_context(tc.tile_pool(name="pmid", bufs=2, space="PSUM"))
            pacc = s1.enter_context(tc.tile_pool(name="pacc", bufs=1, space="PSUM"))
            ptr = s1.enter_context(tc.tile_pool(name="ptr", bufs=2, space="PSUM"))

            def chdim(C):
                T0 = C * 512
                T = 512 if C < NCH2 - 1 else 64
                nsub = T // 128 if C < NCH2 - 1 else 1
                sw = 128 if C < NCH2 - 1 else 64
                return T0, T, nsub, sw

            def front(C):
                T0, T, nsub, sw = chdim(C)
                h3T = sb.tile([128, 2, 512], F32R, tag="h3T", name="h3T")
                mv4 = sb.tile([128, 4, 2], F32, tag="mv4", name="mv4")
                rstd4 = sb.tile([128, 4], F32, tag="rstd4", name="rstd4")
                fxu4 = sb.tile([128, 4, D], F32, tag="fxu4", name="fxu4")
                for s in range(nsub):
                    t0 = T0 + s * 128
                    fps = pmid.tile([128, D], F32, tag="fxu", name="fps")
                    nc.tensor.matmul(fps[0:sw], xt_all[:, t0:t0 + sw], c2pp[:],
                                     start=True, stop=True)
                    nc.vector.tensor_copy(fxu4[0:sw, s, :], fps[0:sw])
                    _ln_stats(nc, sb3, fxu4[0:sw, s, :], sw, mv4[0:sw, s, :])
                _dve_rsqrt(nc, sb3, mv4[0:sw, 0:nsub, 1:2], sw, nsub, rstd4, EPS, magic)
                for s in range(nsub):
                    h3 = sb3.tile([128, D], F32R, tag="h3", name="h3")
                    _ln_apply(nc, h3, fxu4[0:sw, s, :], mv4[0:sw, s, 0:1],
                              rstd4[0:sw, s:s + 1], sw)
                    _transpose_pair(nc, ptr, ident_r, h3, sw,
                                    h3T[:, :, s * 128:s * 128 + sw],
                                    lambda d_, s_: nc.vector.tensor_copy(d_, s_))
                return h3T

            def back(C, h3T):
                T0, T, nsub, sw = chdim(C)
                facc = pacc.tile([128, 4, D], F32, tag="facc", name="facc")
                for fs in range(8):
                    ups = pbig.tile([128, 512], F32, tag="big", name="ups")
                    for dc in range(2):
                        nc.tensor.matmul(ups[:, 0:T], m1[:, dc, fs * 128:(fs + 1) * 128],
                                         h3T[:, dc, 0:T], start=(dc == 0), stop=(dc == 1))
                    uT = sb3.tile([128, 512], F32R, tag="uT", name="uT")
                    nc.scalar.activation(uT[:, 0:T], ups[:, 0:T], AF.Gelu,
                                         bias=ib2[:, fs:fs + 1])
                    for s in range(nsub):
                        nc.tensor.matmul(facc[0:sw, s, :], uT[:, s * 128:s * 128 + sw],
                                         m2[:, fs, :],
                                         start=(fs == 0 and s % 2 == 0),
                                         stop=(fs == 7 and not flags["mb2"]),
                                         skip_group_check=(fs > 0 or s % 2 == 1))
                if flags["mb2"]:
                    for s in range(nsub):
                        nc.tensor.matmul(facc[0:sw, s, :], ones_col[0:1, 0:1].broadcast_to([1, sw]),
                                         mb2[:], start=False, stop=True, skip_group_check=True)
                for s in range(nsub):
                    t0 = T0 + s * 128
                    fo = sb3.tile([128, D], F32, tag="fo", name="fo")
                    nc.vector.tensor_copy(fo[0:sw], facc[0:sw, s, :])
                    nc.sync.dma_start(out=fxo_d[t0:t0 + sw, :], in_=fo[0:sw])

            h3T_c = front(0)
            for C in range(NCH2):
                bk = h3T_c
                h3T_c = front(C + 1) if C + 1 < NCH2 else None
                back(C, bk)

    nc.finalize()
    return nc


_NC_CACHE = {}


def _get_nc(which, flags):
    key = (which, tuple(sorted(flags.items())))
    if key not in _NC_CACHE:
        _NC_CACHE[key] = build_launch1(flags) if which == 1 else build_launch2(flags)
    return _NC_CACHE[key]


def kernel(**inputs):
    inp = {k: np.ascontiguousarray(np.asarray(v)) for k, v in inputs.items()}
    x, fx = inp["x"], inp["fx"]
    f64 = lambda k: inp[k].astype(np.float64)

    # ---- host-side weight folding (LN gains into following weights) ----
    g1, b1 = f64("ln1_g"), f64("ln1_b")
    g2, b2 = f64("ln2_g"), f64("ln2_b")
    g3, b3 = f64("ln3_g"), f64("ln3_b")
    Wq, Wk, Wv = f64("Wq"), f64("Wk"), f64("Wv")
    wqkv = np.concatenate([g1[:, None] * Wq, g1[:, None] * Wk, g1[:, None] * Wv],
                          axis=1).astype(np.float32)
    bqkv = np.concatenate([b1 @ Wq, b1 @ Wk, b1 @ Wv]).astype(np.float32)[None, :]
    w1 = (g2[:, None] * f64("mlp_W1")).astype(np.float32)
    ib1 = (b2 @ f64("mlp_W1") + f64("mlp_b1")).astype(np.float32)
    m1 = (g3[:, None] * f64("mlp2_W1")).astype(np.float32)
    ib2 = (b3 @ f64("mlp2_W1") + f64("mlp2_b1")).astype(np.float32)
    cmask = np.zeros((D, D), np.float32)
    for h in range(H):
        cmask[h * DH:(h + 1) * DH, h * DH:(h + 1) * DH] = DH ** -0.5

    flags1 = {"bqkv": bool(np.any(bqkv)), "bo": bool(np.any(inp["bo"])),
              "b2": bool(np.any(inp["mlp_b2"]))}
    flags1["anybias"] = any(flags1.values()) or bool(np.any(ib1)) or bool(np.any(inp["proj_b1"])) or bool(np.any(inp["proj_b2"]))
    xp = np.zeros((B, NP_, D), np.float32); xp[:, :N] = x
    fxp = np.zeros((B, NP_, D), np.float32); fxp[:, :N] = fx

    flags2 = {"mb2": bool(np.any(inp["mlp2_b2"]))}

    common1 = {
        "wqkv": wqkv, "wo": inp["Wo"], "w1": w1, "w2": inp["mlp_W2"],
        "p1": inp["proj_W1"], "p2": inp["proj_W2"], "cmask": cmask,
        "ib1": ib1, "ip1": inp["proj_b1"], "ipb2": inp["proj_b2"],
    }
    if flags1["bqkv"]:
        common1["bqkv"] = bqkv
    if flags1["bo"]:
        common1["bo"] = inp["bo"][None, :].astype(np.float32)
    if flags1["b2"]:
        common1["b2"] = inp["mlp_b2"][None, :].astype(np.float32)

    nc1 = _get_nc(1, flags1)
    in_maps1 = [dict(common1, x=xp[b], fx=fxp[b]) for b in range(B)]
    res1 = run_bass_kernel_spmd(nc1, in_maps1, CORES).results
    res1 = [{k: np.asarray(v) for k, v in r.items()} for r in res1]

    # ---- host boundary: cov all-reduce + Cholesky + M fold ----
    cov = sum(r["cov"].astype(np.float64) for r in res1) / (B * N)
    L = np.linalg.cholesky(cov)
    Linv = np.linalg.inv(L)
    sp_mu = np.log1p(np.exp(inp["mu"].astype(np.float64)))
    M = Linv.T @ (sp_mu[:, None] * Linv)

    common2 = {"m1": m1, "m2": inp["mlp2_W2"], "ib2": ib2}
    if flags2["mb2"]:
        common2["mb2"] = inp["mlp2_b2"][None, :].astype(np.float32)
    nc2 = _get_nc(2, flags2)
    in_maps2 = [dict(common2, xt=res1[b]["xt"],
                     c2pp=(M @ res1[b]["c2p"].astype(np.float64)).astype(np.float32))
                for b in range(B)]
    res2 = run_bass_kernel_spmd(nc2, in_maps2, CORES).results
    res2 = [{k: np.asarray(v) for k, v in r.items()} for r in res2]

    x_out = np.stack([res1[b]["x2o"][:N] for b in range(B)]).astype(np.float32)
    fx_out = np.stack([res2[b]["fxo"][:N] for b in range(B)]).astype(np.float32)
    return x_out, fx_out



# revision 19
# speedup vs baseline: 1.3076x; 1.3076x over previous
"""TRN2 Bass kernel for nn_ONOBlock (linear attention + MLPs + covariance whitening).

Sharding: data-parallel over batch, 1 batch element per core (B=8, n_cores=8).
Two launches with a host boundary for the [64,64] covariance all-reduce + Cholesky:
  fx_out = X_ @ (L^-T diag(softplus(mu)) L^-1) @ (X_^T fx)

Schedule notes (v2):
 - Pass 1 runs 512-token super-chunks: one x DMA, batched rstd on ACT,
   LN apply on GpSimd.  The k^T v context is accumulated as
   G = [h0 | 1]^T @ ek (token contraction), so the per-sub V projection
   and its PSUM drain disappear; Wv / mask / 1/Z / Wo fold into a
   once-per-pass epilogue producing CW = C @ Wo directly.
 - Pass 2 / launch 2: LayerNorm apply is fused into the PE transpose
   (diag(rstd) moving operand + k=1 rank-1 mean fix); the x+MLP residual
   accumulates into the MLP2 PSUM group as a transpose-matmul, so x2/fx
   are materialized feature-major only and the host de-transposes.
 - Per-chunk stats/rsqrt (serial DVE chain) are emitted a chunk ahead of
   the transposes that consume them, hiding under neighbouring PE work.
 - GpSimd (Pool slot) has no PSUM port: it only gets SBUF->SBUF work;
   PSUM drains are balanced between DVE and ACT.
"""
import contextlib
import numpy as np

import bass_rust as _bass_rust
import concourse.bass as bass
import concourse.bacc as bacc
import concourse.tile as tile
from concourse import mybir
from concourse.hw_specs import get_activation_tables
from concourse.bass_utils import run_bass_kernel_spmd
from concourse.masks import make_identity

class _Bacc(bacc.Bacc):
    """Bacc with act-table selection steered to the combined ln+exp set.

    The stock pass resolves Ln->'natural_log' and Exp->'exp_and_others',
    reloading the ACT table between them (~1.3us each, every chunk).
    Masking those two sets forces both onto 'natural_log_exp_and_others'."""

    def insert_act_table_loads(self):
        has_activation = any(
            isinstance(i, mybir.InstActivation)
            for b in self.main_func.blocks
            for i in b.instructions
        )
        if not has_activation:
            return
        tabs = [
            (nm, (set() if nm in ("natural_log", "exp_and_others") else fs))
            for nm, fs in get_activation_tables(self.m.arch).items()
        ]
        _bass_rust.insert_act_table_loads(self, tabs)


F32 = mybir.dt.float32
F32R = mybir.dt.float32r
I32 = mybir.dt.int32
BF16 = mybir.dt.bfloat16
AF = mybir.ActivationFunctionType
ALU = mybir.AluOpType
AX = mybir.AxisListType

B, N, D, H, PSI = 8, 7225, 256, 8, 64
DH = D // H
DF = 4 * D
EPS = 1e-5
NP_ = 7232            # padded sequence: 56*128 + 64
NCH2 = 15             # 512-token chunks (14 of 512 + 1 of 64)
CORES = list(range(8))


def _bcast(ap, parts):
    """Free-dim broadcast helper: [p, g] -> [p, g, parts] with 0-stride."""
    return bass.AP(tensor=ap.tensor, offset=ap.offset,
                   ap=[ap.ap[0], ap.ap[1], [0, parts]])


def _dup2(ap):
    """[1, w] -> [1, 2, w] 0-stride duplication (for rank-1 matmul rhs)."""
    return bass.AP(tensor=ap.tensor, offset=ap.offset,
                   ap=[ap.ap[0], [0, 2], ap.ap[1]])


def _ln_stats(nc, pool, x_ap, w, mv_slot):
    """bn stats into mv_slot [w, 2] = (mean, var)."""
    stats = pool.tile([128, 6], F32, tag="ln_stats")
    nc.vector.bn_stats(out=stats[0:w], in_=x_ap)
    nc.vector.bn_aggr(out=mv_slot, in_=stats[0:w])


def _dve_rsqrt(nc, pool, var_ap, w, n, rstd_out, eps, magic):
    """rstd_out[0:w, 0:n] = 1/sqrt(var_ap + eps) entirely on DVE.

    Quake bit-trick init + 2 Newton steps; ~1e-5 rel err."""
    v4 = pool.tile([128, 4], F32, tag="rs_v")
    nc.vector.tensor_scalar(out=v4[0:w, 0:n], in0=var_ap, scalar1=float(eps),
                            scalar2=None, op0=ALU.add)
    sh = pool.tile([128, 4], I32, tag="rs_sh")
    nc.vector.tensor_scalar(out=sh[0:w, 0:n], in0=v4[0:w, 0:n].bitcast(I32),
                            scalar1=1, scalar2=None, op0=ALU.logical_shift_right)
    y = rstd_out
    nc.vector.tensor_tensor(out=y[0:w, 0:n].bitcast(I32), in0=magic[0:w, 0:n],
                            in1=sh[0:w, 0:n], op=ALU.subtract)
    t = pool.tile([128, 4], F32, tag="rs_t")
    for _ in range(2):
        nc.vector.tensor_tensor(out=t[0:w, 0:n], in0=y[0:w, 0:n], in1=y[0:w, 0:n], op=ALU.mult)
        nc.vector.tensor_tensor(out=t[0:w, 0:n], in0=t[0:w, 0:n], in1=v4[0:w, 0:n], op=ALU.mult)
        nc.vector.tensor_scalar(out=t[0:w, 0:n], in0=t[0:w, 0:n], scalar1=-0.5,
                                scalar2=1.5, op0=ALU.mult, op1=ALU.add)
        nc.vector.tensor_tensor(out=y[0:w, 0:n], in0=y[0:w, 0:n], in1=t[0:w, 0:n], op=ALU.mult)


def chdim(C):
    T0 = C * 512
    T = 512 if C < NCH2 - 1 else 64
    nsub = T // 128 if C < NCH2 - 1 else 1
    sw = 128 if C < NCH2 - 1 else 64
    return T0, T, nsub, sw


def build_launch1(flags):
    nc = _Bacc(None)
    # ---- I/O ----
    x_d = nc.dram_tensor("x", [NP_, D], F32, kind="ExternalInput")
    fx_d = nc.dram_tensor("fx", [NP_, D], F32R, kind="ExternalInput")
    wqkv_d = nc.dram_tensor("wqkv", [D, 3 * D], F32R, kind="ExternalInput")
    wo_d = nc.dram_tensor("wo", [D, D], F32R, kind="ExternalInput")
    w1_d = nc.dram_tensor("w1", [D, DF], F32R, kind="ExternalInput")
    w2_d = nc.dram_tensor("w2", [DF, D], F32R, kind="ExternalInput")
    p1_d = nc.dram_tensor("p1", [D, D], F32R, kind="ExternalInput")
    p2_d = nc.dram_tensor("p2", [D, PSI], F32R, kind="ExternalInput")
    cmask_d = nc.dram_tensor("cmask", [D, D], F32, kind="ExternalInput")
    ib1_d = nc.dram_tensor("ib1", [DF], F32, kind="ExternalInput")
    ip1_d = nc.dram_tensor("ip1", [D], F32, kind="ExternalInput")
    ipb2_d = nc.dram_tensor("ipb2", [PSI], F32, kind="ExternalInput")
    if flags["bqkv"]:
        bqkv_d = nc.dram_tensor("bqkv", [1, 3 * D], F32R, kind="ExternalInput")
    if flags["bo"]:
        bo_d = nc.dram_tensor("bo", [1, D], F32R, kind="ExternalInput")
    if flags["b2"]:
        b2_d = nc.dram_tensor("b2", [1, D], F32R, kind="ExternalInput")

    x2o_d = nc.dram_tensor("x2o", [2, 128, NP_], F32, kind="ExternalOutput")
    xt_d = nc.dram_tensor("xt", [PSI, NP_], F32, kind="ExternalOutput")
    cov_d = nc.dram_tensor("cov", [PSI, PSI], F32, kind="ExternalOutput")
    c2p_d = nc.dram_tensor("c2p", [PSI, D], F32, kind="ExternalOutput")

    with tile.TileContext(nc) as tc, contextlib.ExitStack() as top:
        wp = top.enter_context(tc.tile_pool(name="wp", bufs=1))
        # ---- resident weights/constants ----
        # pass-1-critical weights first; bulky pass-2 weights are issued
        # last so the first x chunks don't queue behind ~4.5MB of DMA.
        wqkv = wp.tile([128, 2, 3 * D], F32R)
        nc.sync.dma_start(out=wqkv, in_=wqkv_d.rearrange("(c p) e -> p c e", p=128))
        wo = wp.tile([128, 2, D], F32R)
        nc.sync.dma_start(out=wo, in_=wo_d.rearrange("(c p) e -> p c e", p=128))
        cmask = wp.tile([128, 2, D], F32)
        nc.sync.dma_start(out=cmask, in_=cmask_d.rearrange("(c p) e -> p c e", p=128))
        ib1 = wp.tile([128, 8], F32)
        nc.sync.dma_start(out=ib1, in_=ib1_d.rearrange("(a p) -> p a", p=128))
        ip1 = wp.tile([128, 2], F32)
        nc.sync.dma_start(out=ip1, in_=ip1_d.rearrange("(a p) -> p a", p=128))
        ipb2 = wp.tile([64, 1], F32)
        nc.sync.dma_start(out=ipb2, in_=ipb2_d.rearrange("(p a) -> p a", a=1))
        w1 = wp.tile([128, 2, DF], F32R)
        w2 = wp.tile([128, 8, D], F32R)
        p1 = wp.tile([128, 2, D], F32R)
        p2 = wp.tile([128, 2, PSI], F32R)

        def _load_pass2_weights():
            nc.sync.dma_start(out=w1, in_=w1_d.rearrange("(c p) e -> p c e", p=128))
            nc.sync.dma_start(out=w2, in_=w2_d.rearrange("(c p) e -> p c e", p=128))
            nc.sync.dma_start(out=p1, in_=p1_d.rearrange("(c p) e -> p c e", p=128))
            nc.sync.dma_start(out=p2, in_=p2_d.rearrange("(c p) e -> p c e", p=128))
        if flags["bqkv"]:
            bqkv = wp.tile([1, 3 * D], F32R)
            nc.sync.dma_start(out=bqkv, in_=bqkv_d[:])
        if flags["bo"]:
            bo = wp.tile([1, D], F32R)
            nc.sync.dma_start(out=bo, in_=bo_d[:])
        if flags["b2"]:
            b2 = wp.tile([1, D], F32R)
            nc.sync.dma_start(out=b2, in_=b2_d[:])

        eps_t = wp.tile([128, 1], F32)
        nc.vector.memset(eps_t, EPS)
        magic = wp.tile([128, 4], I32)
        nc.vector.memset(magic, 0x5F3759DF)
        ident = wp.tile([128, 128], F32)
        make_identity(nc, ident)
        ident_r = wp.tile([128, 128], F32R)
        nc.vector.tensor_copy(ident_r, ident)
        ident4 = wp.tile([128, 4, 128], F32R)
        for s in range(4):
            nc.vector.tensor_copy(ident4[:, s, :], ident_r)
        ones_f = wp.tile([128, 16], F32)
        nc.vector.memset(ones_f, 1.0)
        ones_col = wp.tile([128, 1], F32R)
        nc.vector.tensor_copy(ones_col, ones_f[:, 0:1])
        nones_f = wp.tile([1, 128], F32)
        nc.vector.memset(nones_f, -1.0)
        nones = wp.tile([1, 128], F32R)
        nc.vector.tensor_copy(nones, nones_f)
        onesr_row = wp.tile([1, 512], F32R)
        if flags["b2"]:
            of = wp.tile([1, 512], F32)
            nc.vector.memset(of, 1.0)
            nc.vector.tensor_copy(onesr_row, of)
        zero_f = wp.tile([128, 16], F32)
        nc.vector.memset(zero_f, 0.0)

        qT = wp.tile([128, 2, NP_], BF16)      # q softmax'd, transposed, resident
        CW_sb = wp.tile([128, 2, D], F32R)     # C @ Wo

        # ================= PASS 1 =================
        # 512-token super-chunks, 4 subs of 128 tokens each.
        with contextlib.ExitStack() as s1:
            sb = s1.enter_context(tc.tile_pool(name="p1sb", bufs=3))
            sb3 = s1.enter_context(tc.tile_pool(name="p1sb3", bufs=4))
            sb6 = s1.enter_context(tc.tile_pool(name="p1sb6", bufs=6))
            pctx = s1.enter_context(tc.tile_pool(name="pctx", bufs=1, space="PSUM"))
            pqk = s1.enter_context(tc.tile_pool(name="pqk", bufs=3, space="PSUM"))
            ptr = s1.enter_context(tc.tile_pool(name="ptr", bufs=2, space="PSUM"))

            # G = [h0 | 1]^T @ ek accumulators: two 128-row tiles + Z row
            g_ps = [pctx.tile([128, 256], F32, tag=f"g{i}", name=f"g_ps{i}")
                    for i in range(2)]
            z_ps = pctx.tile([2, 256], F32, tag="gz", name="z_ps")

            def p1_dma(C):
                T0, T, nsub, sw = chdim(C)
                x4 = sb.tile([128, 4, D], F32, tag="x4", name="x4")
                nc.sync.dma_start(
                    out=x4[0:sw, 0:nsub],
                    in_=x_d[T0:T0 + T].rearrange("(s p) e -> p s e", p=sw))
                return x4

            def p1_front_gen(C, x4):
                """LN stats + batched rstd + apply(GP) + h0 transposes."""
                T0, T, nsub, sw = chdim(C)
                mv4 = sb3.tile([128, 4, 2], F32, tag="mv4", name="mv4")
                rstd4 = sb3.tile([128, 4], F32, tag="rstd4", name="rstd4")
                for s in range(nsub):
                    _ln_stats(nc, sb3, x4[0:sw, s, :], sw, mv4[0:sw, s, :])
                    yield
                nc.scalar.activation(
                    rstd4[0:sw, 0:nsub],
                    mv4[0:sw, 0:nsub, 1:2].rearrange("p s a -> p (s a)"),
                    AF.Ln, bias=eps_t[0:sw])
                nc.scalar.activation(rstd4[0:sw, 0:nsub], rstd4[0:sw, 0:nsub],
                                     AF.Exp, scale=-0.5)
                yield
                h0T = sb.tile([128, 2, 512], F32R, tag="h0T", name="h0T")
                h0s = []
                for s in range(nsub):
                    h0 = sb6.tile([128, 264], F32R, tag="h0", name="h0")
                    nc.gpsimd.tensor_scalar(out=h0[0:sw, 0:D], in0=x4[0:sw, s, :],
                                            scalar1=mv4[0:sw, s, 0:1],
                                            scalar2=rstd4[0:sw, s:s + 1],
                                            op0=ALU.subtract, op1=ALU.mult)
                    nc.vector.tensor_copy(h0[0:sw, D:D + 1], ones_f[0:sw, 0:1].bitcast(F32R))
                    pt = ptr.tile([128, 256], F32R, tag="tr", name="pt")
                    for dc in range(2):
                        nc.tensor.matmul(pt[:, dc * sw:(dc + 1) * sw],
                                         h0[0:sw, dc * 128:(dc + 1) * 128],
                                         ident_r[0:sw, 0:sw], is_transpose=True,
                                         skip_group_check=(dc == 1))
                    if s % 2 == 0:
                        nc.vector.tensor_copy(h0T[:, :, s * 128:s * 128 + sw],
                                              pt[:, 0:2 * sw].rearrange("p (c w) -> p c w", c=2))
                    else:
                        nc.scalar.activation(h0T[:, :, s * 128:s * 128 + sw],
                                             pt[:, 0:2 * sw].rearrange("p (c w) -> p c w", c=2),
                                             AF.Copy)
                    h0s.append(h0)
                    yield
                return (h0T, h0s, (T0, T, nsub, sw))

            def p1_qk(st, s):
                h0T, h0s, (T0, T, nsub, sw) = st
                ps_qk = pqk.tile([128, 2 * D], F32, tag="qk", name="ps_qk")
                for i in range(2):
                    for dc in range(2):
                        nc.tensor.matmul(ps_qk[0:sw, i * D:(i + 1) * D],
                                         h0T[:, dc, s * 128:s * 128 + sw],
                                         wqkv[:, dc, i * D:(i + 1) * D],
                                         start=(dc == 0),
                                         stop=(dc == 1 and not flags["bqkv"]),
                                         skip_group_check=(i == 1))
                if flags["bqkv"]:
                    nc.tensor.matmul(ps_qk[0:sw, 0:2 * D],
                                     ones_col[0:1, 0:1].broadcast_to([1, sw]),
                                     bqkv[:, 0:2 * D], start=False, stop=True,
                                     skip_group_check=True)
                return ps_qk

            def p1_back(st, s, ps_qk):
                h0T, h0s, (T0, T, nsub, sw) = st
                t0 = T0 + s * 128
                w = sw
                eqk = sb3.tile([128, 2 * D], F32R, tag="eqk", name="eqk")
                nc.scalar.activation(eqk[0:w], ps_qk[0:w], AF.Exp)
                eq = eqk[:, 0:D]
                ek = eqk[:, D:2 * D]
                qs = sb3.tile([128, 8], F32, tag="qs", name="qs")
                nc.vector.reduce_sum(out=qs[0:w],
                                     in_=eq[0:w].rearrange("p (g s) -> p g s", g=8),
                                     axis=AX.X)
                nc.vector.reciprocal(qs[0:w], qs[0:w])
                q_sm = sb3.tile([128, D], F32R, tag="q_sm", name="q_sm")
                nc.gpsimd.tensor_tensor(out=q_sm[0:w].rearrange("p (g s) -> p g s", g=8),
                                        in0=eq[0:w].rearrange("p (g s) -> p g s", g=8),
                                        in1=_bcast(qs[0:w], 32), op=ALU.mult)
                ptq = ptr.tile([128, 256], F32R, tag="tr", name="ptq")
                for dc in range(2):
                    nc.tensor.matmul(ptq[:, dc * w:(dc + 1) * w],
                                     q_sm[0:w, dc * 128:(dc + 1) * 128],
                                     ident_r[0:w, 0:w], is_transpose=True,
                                     skip_group_check=(dc == 1))
                nc.scalar.activation(qT[:, :, t0:t0 + w],
                                     ptq[:, 0:2 * w].rearrange("p (c w) -> p c w", c=2),
                                     AF.Copy)

                # G += [h0 | 1]^T @ ek  (token contraction, clamped to real N)
                h0 = h0s[s]
                kv = w if t0 + w <= N else max(0, N - t0)
                first = (t0 == 0)
                last = (t0 + w >= NP_)
                for i in range(2):
                    nc.tensor.matmul(g_ps[i][:, 0:D],
                                     h0[0:kv, i * 128:(i + 1) * 128],
                                     ek[0:kv], start=first, stop=last)
                nc.tensor.matmul(z_ps[0:1, 0:D], h0[0:kv, D:D + 1],
                                 ek[0:kv], start=first, stop=last)

            NCH1 = NCH2
            x4q = {0: p1_dma(0), 1: p1_dma(1)}
            _load_pass2_weights()
            st = _Step(p1_front_gen(0, x4q.pop(0))).run()
            fg = _Step(p1_front_gen(1, x4q.pop(1)))
            pend = []
            for C in range(NCH1):
                T0, T, nsub, sw = chdim(C)
                if C + 2 < NCH1:
                    x4q[C + 2] = p1_dma(C + 2)
                for s in range(nsub):
                    qk = p1_qk(st, s)
                    pend.append((st, s, qk))
                    if len(pend) >= 3:
                        a = pend.pop(0)
                        p1_back(a[0], a[1], a[2])
                    if fg is not None:
                        fg.step()
                        fg.step()
                if fg is not None:
                    st = fg.run()
                    fg = (_Step(p1_front_gen(C + 2, x4q.pop(C + 2)))
                          if C + 2 < NCH1 else None)
            while pend:
                a = pend.pop(0)
                p1_back(a[0], a[1], a[2])

            # ---- epilogue: CW = (mask . (G^T Wv-ctx)) / Z @ Wo, once ----
            g_sb = sb3.tile([128, 2, 256], F32R, tag="g_sb", name="g_sb")
            nc.vector.tensor_copy(g_sb[:, 0, :], g_ps[0][:, 0:D])
            nc.vector.tensor_copy(g_sb[:, 1, :], g_ps[1][:, 0:D])
            # Z row [1, 256] -> per-partition column [128, 2] via k=1 matmul
            z_row = sb3.tile([1, 256], F32R, tag="zrow", name="z_row")
            nc.vector.tensor_copy(z_row[0:1], z_ps[0:1, 0:D])
            ones2 = sb3.tile([1, 2], F32R, tag="ones2", name="ones2")
            nc.vector.tensor_copy(ones2[0:1], ones_f[0:1, 0:2].bitcast(F32R))
            zcol = sb3.tile([128, 2], F32, tag="zcol", name="zcol")
            for m in range(2):
                ztr = ptr.tile([128, 256], F32, tag="tr", name="ztr")
                nc.tensor.matmul(ztr[0:128, 0:2],
                                 z_row[0:1, m * 128:(m + 1) * 128],
                                 ones2[0:1, 0:2], start=True, stop=True)
                nc.vector.tensor_copy(zcol[:, m:m + 1], ztr[0:128, 0:1])
            nc.vector.reciprocal(zcol, zcol)
            # ctxW[e, f] = Wv^T G : masked on drain
            ctxm = sb3.tile([128, 2, 256], F32R, tag="ctxm", name="ctxm")
            for es in range(2):
                cps = pqk.tile([128, 2 * D], F32, tag="qk", name="cps")
                for dc in range(2):
                    nc.tensor.matmul(cps[:, 0:D], wqkv[:, dc, 2 * D + es * 128:2 * D + (es + 1) * 128],
                                     g_sb[:, dc, :], start=(dc == 0), stop=(dc == 1))
                nc.vector.tensor_tensor(out=ctxm[:, es, :], in0=cps[:, 0:D],
                                        in1=cmask[:, es, :].bitcast(F32R), op=ALU.mult)
            # CW_raw[f, d] = (mask.ctx)[f, e] Wo[e, d]; scale rows by 1/Z
            for m in range(2):
                cwps = pqk.tile([128, 2 * D], F32, tag="qk", name="cwps")
                for ec in range(2):
                    nc.tensor.matmul(cwps[:, 0:D], ctxm[:, ec, m * 128:(m + 1) * 128],
                                     wo[:, ec, :], start=(ec == 0), stop=(ec == 1))
                nc.vector.tensor_scalar(out=CW_sb[:, m, :], in0=cwps[:, 0:D],
                                        scalar1=zcol[:, m:m + 1], scalar2=None,
                                        op0=ALU.mult)
            # zero the padded q columns (exp(0)=1 otherwise)
            for dc in range(2):
                nc.vector.memset(qT[:, dc, N:NP_], 0.0)

        # ================= PASS 2 =================
        with contextlib.ExitStack() as s2:
            sb = s2.enter_context(tc.tile_pool(name="p2sb", bufs=3))
            sb3 = s2.enter_context(tc.tile_pool(name="p2sb3", bufs=4))
            sbd = s2.enter_context(tc.tile_pool(name="p2dma", bufs=2))
            pcc = s2.enter_context(tc.tile_pool(name="pcc", bufs=1, space="PSUM"))
            pbig = s2.enter_context(tc.tile_pool(name="pbig", bufs=3, space="PSUM"))
            px2 = s2.enter_context(tc.tile_pool(name="px2", bufs=1, space="PSUM"))
            ptr = s2.enter_context(tc.tile_pool(name="ptr2", bufs=2, space="PSUM"))

            cc_ps = pcc.tile([64, 320], F32)

            def p2_dma_x(C):
                T0, T, nsub, sw = chdim(C)
                x_in4 = sbd.tile([128, 4, D], F32, tag="xin4", name="x_in4")
                nc.sync.dma_start(
                    out=x_in4[0:sw, 0:nsub],
                    in_=x_d[T0:T0 + T].rearrange("(s p) e -> p s e", p=sw))
                return x_in4

            def p2_dma_fx(C):
                T0, T, nsub, sw = chdim(C)
                fx4 = sbd.tile([128, 4, D], F32R, tag="fx4", name="fx4")
                nc.sync.dma_start(
                    out=fx4[0:sw, 0:nsub],
                    in_=fx_d[T0:T0 + T].rearrange("(s p) e -> p s e", p=sw))
                return fx4

            def front_a(C, x_in4):
                """attn apply + residual(PE) + LN stats + rsqrt."""
                T0, T, nsub, sw = chdim(C)
                x1_sb = sb.tile([128, 4, D], F32R, tag="x1", name="x1_sb")
                mv4 = sb.tile([128, 4, 2], F32, tag="mv4", name="mv4")
                rstd4 = sb.tile([128, 4], F32, tag="rstd4", name="rstd4")
                mr4 = sb.tile([128, 4], F32R, tag="mr4", name="mr4")
                for s in range(nsub):
                    t0 = T0 + s * 128
                    xps = pbig.tile([128, 512], F32, tag="big", name="xps")
                    for dc in range(2):
                        nc.tensor.matmul(xps[0:sw, 0:D], qT[:, dc, t0:t0 + sw],
                                         CW_sb[:, dc, :],
                                         start=(dc == 0), stop=False)
                    if flags["bo"]:
                        nc.tensor.matmul(xps[0:sw, 0:D],
                                         ones_col[0:1, 0:1].broadcast_to([1, sw]),
                                         bo[:], start=False, stop=False,
                                         skip_group_check=True)
                    nc.tensor.matmul(xps[0:sw, 0:D], ident_r[0:sw, 0:sw],
                                     x_in4[0:sw, s, :].bitcast(F32R),
                                     start=False, stop=True, skip_group_check=True)
                    if s % 2 == 0:
                        nc.vector.tensor_copy(x1_sb[0:sw, s, :], xps[0:sw, 0:D])
                    else:
                        nc.scalar.activation(x1_sb[0:sw, s, :], xps[0:sw, 0:D], AF.Copy)
                    _ln_stats(nc, sb3, x1_sb[0:sw, s, :].bitcast(F32), sw, mv4[0:sw, s, :])
                _dve_rsqrt(nc, sb3, mv4[0:sw, 0:nsub, 1:2], sw, nsub, rstd4, EPS, magic)
                nc.vector.tensor_tensor(
                    out=mr4[0:sw, 0:nsub],
                    in0=mv4[0:sw, 0:nsub, 0:1].rearrange("p s a -> p (s a)"),
                    in1=rstd4[0:sw, 0:nsub].bitcast(F32R), op=ALU.mult)
                return (x1_sb, mv4, rstd4, mr4, (T0, T, nsub, sw))

            def front_b(st):
                """diag build + LN-fused transposes -> h2T."""
                x1_sb, mv4, rstd4, mr4, (T0, T, nsub, sw) = st
                h2T = sb.tile([128, 2, 512], F32R, tag="h2T", name="h2T")
                diag4 = sb3.tile([128, 4, 128], F32R, tag="diag4", name="diag4")
                for s in range(nsub):
                    nc.gpsimd.tensor_scalar(out=diag4[0:sw, s, 0:sw],
                                            in0=ident4[0:sw, s, 0:sw],
                                            scalar1=rstd4[0:sw, s:s + 1],
                                            scalar2=None, op0=ALU.mult)
                mrp = pbig.tile([128, 512], F32R, tag="big", name="mrp")
                for s in range(nsub):
                    nc.tensor.matmul(mrp[0:1, s * sw:(s + 1) * sw],
                                     mr4[0:sw, s:s + 1],
                                     ident_r[0:sw, 0:sw], is_transpose=True,
                                     skip_group_check=(s > 0))
                mrow = sb3.tile([1, 512], F32R, tag="mrow", name="mrow")
                nc.vector.tensor_copy(mrow[0:1, 0:nsub * sw], mrp[0:1, 0:nsub * sw])
                for s in range(nsub):
                    pt = ptr.tile([128, 256], F32R, tag="tr", name="pt")
                    for dc in range(2):
                        nc.tensor.matmul(pt[:, dc * sw:(dc + 1) * sw],
                                         x1_sb[0:sw, s, dc * 128:(dc + 1) * 128],
                                         diag4[0:sw, s, 0:sw], is_transpose=True,
                                         start=True, stop=False,
                                         skip_group_check=(dc == 1))
                    nc.tensor.matmul(pt[:, 0:2 * sw].bitcast(F32), nones[0:1, 0:128],
                                     _dup2(mrow[0:1, s * sw:(s + 1) * sw]),
                                     start=False, stop=True, skip_group_check=True)
                    if s % 2 == 0:
                        nc.vector.tensor_copy(h2T[:, :, s * 128:s * 128 + sw],
                                              pt[:, 0:2 * sw].rearrange("p (c w) -> p c w", c=2))
                    else:
                        nc.scalar.activation(h2T[:, :, s * 128:s * 128 + sw],
                                             pt[:, 0:2 * sw].rearrange("p (c w) -> p c w", c=2),
                                             AF.Copy)
                return h2T

            def back_mlp(C, st, h2T):
                """residual transpose into PSUM + MLP (transposed x2 out)."""
                x1_sb, mv4, rstd4, mr4, (T0, T, nsub, sw) = st
                x2Tps = px2.tile([128, 2, 512], F32, tag="x2acc", name="x2Tps")
                for s in range(nsub):
                    for dc in range(2):
                        nc.tensor.matmul(x2Tps[:, dc, s * 128:s * 128 + sw].bitcast(F32R),
                                         x1_sb[0:sw, s, dc * 128:(dc + 1) * 128],
                                         ident_r[0:sw, 0:sw], is_transpose=True,
                                         start=True, stop=False,
                                         skip_group_check=(s > 0 or dc > 0))
                uq = []
                def x2mm(fs, uT):
                    for dc in range(2):
                        nc.tensor.matmul(x2Tps[:, dc, 0:T],
                                         w2[:, fs, dc * 128:(dc + 1) * 128],
                                         uT[:, 0:T], start=False,
                                         stop=(fs == 7 and not flags["b2"]),
                                         skip_group_check=True)
                for fs in range(8):
                    ups = pbig.tile([128, 512], F32, tag="big", name="ups")
                    for dc in range(2):
                        nc.tensor.matmul(ups[:, 0:T], w1[:, dc, fs * 128:(fs + 1) * 128],
                                         h2T[:, dc, 0:T], start=(dc == 0), stop=(dc == 1))
                    uT = sb3.tile([128, 512], F32R, tag="uT", name="uT")
                    nc.scalar.activation(uT[:, 0:T], ups[:, 0:T], AF.Gelu,
                                         bias=ib1[:, fs:fs + 1])
                    uq.append((fs, uT))
                    if len(uq) >= 3:
                        x2mm(*uq.pop(0))
                while uq:
                    x2mm(*uq.pop(0))
                if flags["b2"]:
                    for dc in range(2):
                        nc.tensor.matmul(x2Tps[:, dc, 0:T],
                                         b2[:, dc * 128:(dc + 1) * 128],
                                         onesr_row[0:1, 0:T], start=False,
                                         stop=True, skip_group_check=True)
                return x2Tps

            def back_tail(C, st, x2Tps, fx4):
                x1_sb, mv4, rstd4, mr4, (T0, T, nsub, sw) = st
                x2T = sb.tile([128, 2, 512], F32R, tag="x2T", name="x2T")
                nc.scalar.activation(x2T[:, 0, 0:T], x2Tps[:, 0, 0:T], AF.Copy)
                nc.vector.tensor_copy(x2T[:, 1, 0:T], x2Tps[:, 1, 0:T])
                nc.sync.dma_start(out=x2o_d[:, :, T0:T0 + T].rearrange("c p t -> p c t"),
                                  in_=x2T[:, :, 0:T].bitcast(F32))

                pT = sb.tile([128, 2, 512], F32R, tag="pT", name="pT")
                for pc in range(2):
                    pps = pbig.tile([128, 512], F32, tag="big", name="pps")
                    for dc in range(2):
                        nc.tensor.matmul(pps[:, 0:T], p1[:, dc, pc * 128:(pc + 1) * 128],
                                         x2T[:, dc, 0:T], start=(dc == 0), stop=(dc == 1))
                    nc.scalar.activation(pT[:, pc, 0:T], pps[:, 0:T], AF.Gelu,
                                         bias=ip1[:, pc:pc + 1])
                xtps = pbig.tile([128, 512], F32, tag="big", name="xtps")
                for pc in range(2):
                    nc.tensor.matmul(xtps[0:64, 0:T], p2[:, pc, :], pT[:, pc, 0:T],
                                     start=(pc == 0), stop=(pc == 1))
                xT_sb = sb.tile([64, 512], F32R, tag="xT_sb", name="xT_sb")
                if flags["pb2"]:
                    nc.scalar.activation(xT_sb[:, 0:T], xtps[0:64, 0:T], AF.Identity,
                                         bias=ipb2[:, 0:1])
                else:
                    nc.vector.tensor_copy(xT_sb[:, 0:T], xtps[0:64, 0:T])
                nc.sync.dma_start(out=xt_d[:, T0:T0 + T], in_=xT_sb[:, 0:T].bitcast(F32))

                for s in range(nsub):
                    t0 = T0 + s * 128
                    vv = min(sw, N - t0)
                    xtr = ptr.tile([128, 256], F32R, tag="tr", name="xtr")
                    nc.tensor.transpose(xtr[0:sw, 0:64], xT_sb[:, s * 128:s * 128 + sw],
                                        ident_r[0:64, 0:64])
                    xcol = sb3.tile([128, 64], F32R, tag="xcol", name="xcol")
                    if vv < sw and flags.get("anybias"):
                        nc.vector.memset(xcol[0:sw], 0.0)
                        nc.vector.tensor_copy(xcol[0:vv], xtr[0:vv, 0:64])
                    else:
                        nc.vector.tensor_copy(xcol[0:sw], xtr[0:sw, 0:64])
                    last = (C == NCH2 - 1 and s == nsub - 1)
                    first = (C == 0 and s == 0)
                    nc.tensor.matmul(cc_ps[:, 0:64], xcol[0:sw], xcol[0:sw],
                                     start=first, stop=last)
                    nc.tensor.matmul(cc_ps[:, 64:320], xcol[0:sw], fx4[0:sw, s, :],
                                     start=first, stop=last, skip_group_check=True)

            # software pipeline: stats(C+1) overlap tail(C); transposes(C+1)
            # after tail(C) PE work.  DMAs prefetched one stage earlier.
            xin_c = p2_dma_x(0)
            st = front_a(0, xin_c)
            h2T_c = front_b(st)
            fx_c = p2_dma_fx(0)
            for C in range(NCH2):
                if C + 1 < NCH2:
                    xin_n = p2_dma_x(C + 1)
                    fx_n = p2_dma_fx(C + 1)
                x2Tps = back_mlp(C, st, h2T_c)
                nxt = front_a(C + 1, xin_n) if C + 1 < NCH2 else None
                back_tail(C, st, x2Tps, fx_c)
                if nxt is not None:
                    h2T_c = front_b(nxt)
                    st = nxt
                    fx_c = fx_n

            cc_sb = sb.tile([64, 320], F32, tag="cc_sb")
            nc.vector.tensor_copy(cc_sb, cc_ps)
            nc.sync.dma_start(out=cov_d[:], in_=cc_sb[:, 0:64])
            nc.sync.dma_start(out=c2p_d[:], in_=cc_sb[:, 64:320])

    nc.finalize()
    return nc


def build_launch2(flags):
    nc = _Bacc(None)
    xt_d = nc.dram_tensor("xt", [PSI, NP_], F32R, kind="ExternalInput")
    c2pp_d = nc.dram_tensor("c2pp", [PSI, D], F32R, kind="ExternalInput")
    m1_d = nc.dram_tensor("m1", [D, DF], F32R, kind="ExternalInput")
    m2_d = nc.dram_tensor("m2", [DF, D], F32R, kind="ExternalInput")
    ib2_d = nc.dram_tensor("ib2", [DF], F32, kind="ExternalInput")
    if flags["mb2"]:
        mb2_d = nc.dram_tensor("mb2", [1, D], F32R, kind="ExternalInput")
    fxo_d = nc.dram_tensor("fxo", [2, 128, NP_], F32, kind="ExternalOutput")

    with tile.TileContext(nc) as tc, contextlib.ExitStack() as top:
        wp = top.enter_context(tc.tile_pool(name="wp", bufs=1))
        xt_all = wp.tile([64, NP_], F32R)
        nc.sync.dma_start(out=xt_all, in_=xt_d[:])
        c2pp = wp.tile([64, D], F32R)
        nc.sync.dma_start(out=c2pp, in_=c2pp_d[:])
        m1 = wp.tile([128, 2, DF], F32R)
        nc.sync.dma_start(out=m1, in_=m1_d.rearrange("(c p) e -> p c e", p=128))
        m2 = wp.tile([128, 8, D], F32R)
        nc.sync.dma_start(out=m2, in_=m2_d.rearrange("(c p) e -> p c e", p=128))
        ib2 = wp.tile([128, 8], F32)
        nc.sync.dma_start(out=ib2, in_=ib2_d.rearrange("(a p) -> p a", p=128))
        onesr_row = wp.tile([1, 512], F32R)
        if flags["mb2"]:
            mb2 = wp.tile([1, D], F32R)
            nc.sync.dma_start(out=mb2, in_=mb2_d[:])
            of = wp.tile([1, 512], F32)
            nc.vector.memset(of, 1.0)
            nc.vector.tensor_copy(onesr_row, of)
        eps_t = wp.tile([128, 1], F32)
        nc.vector.memset(eps_t, EPS)
        magic = wp.tile([128, 4], I32)
        nc.vector.memset(magic, 0x5F3759DF)
        ident = wp.tile([128, 128], F32)
        make_identity(nc, ident)
        ident_r = wp.tile([128, 128], F32R)
        nc.vector.tensor_copy(ident_r, ident)
        ident4 = wp.tile([128, 4, 128], F32R)
        for s in range(4):
            nc.vector.tensor_copy(ident4[:, s, :], ident_r)
        nones_f = wp.tile([1, 128], F32)
        nc.vector.memset(nones_f, -1.0)
        nones = wp.tile([1, 128], F32R)
        nc.vector.tensor_copy(nones, nones_f)

        with contextlib.ExitStack() as s1:
            sb = s1.enter_context(tc.tile_pool(name="sb", bufs=3))
            sb3 = s1.enter_context(tc.tile_pool(name="sb3", bufs=4))
            pbig = s1.enter_context(tc.tile_pool(name="pbig", bufs=2, space="PSUM"))
            pmid = s1.enter_context(tc.tile_pool(name="pmid", bufs=2, space="PSUM"))
            pfo = s1.enter_context(tc.tile_pool(name="pfo", bufs=1, space="PSUM"))
            ptr = s1.enter_context(tc.tile_pool(name="ptr", bufs=2, space="PSUM"))

            def front_a(C):
                T0, T, nsub, sw = chdim(C)
                fxu4 = sb.tile([128, 4, D], F32R, tag="fxu4", name="fxu4")
                mv4 = sb.tile([128, 4, 2], F32, tag="mv4", name="mv4")
                rstd4 = sb.tile([128, 4], F32, tag="rstd4", name="rstd4")
                mr4 = sb.tile([128, 4], F32R, tag="mr4", name="mr4")
                for s in range(nsub):
                    t0 = T0 + s * 128
                    fps = pmid.tile([128, D], F32, tag="fxu", name="fps")
                    nc.tensor.matmul(fps[0:sw], xt_all[:, t0:t0 + sw], c2pp[:],
                                     start=True, stop=True)
                    if s % 2 == 0:
                        nc.vector.tensor_copy(fxu4[0:sw, s, :], fps[0:sw])
                    else:
                        nc.scalar.activation(fxu4[0:sw, s, :], fps[0:sw], AF.Copy)
                    _ln_stats(nc, sb3, fxu4[0:sw, s, :].bitcast(F32), sw, mv4[0:sw, s, :])
                _dve_rsqrt(nc, sb3, mv4[0:sw, 0:nsub, 1:2], sw, nsub, rstd4, EPS, magic)
                nc.vector.tensor_tensor(
                    out=mr4[0:sw, 0:nsub],
                    in0=mv4[0:sw, 0:nsub, 0:1].rearrange("p s a -> p (s a)"),
                    in1=rstd4[0:sw, 0:nsub].bitcast(F32R), op=ALU.mult)
                return (fxu4, mv4, rstd4, mr4, (T0, T, nsub, sw))

            def front_b(st):
                fxu4, mv4, rstd4, mr4, (T0, T, nsub, sw) = st
                h3T = sb.tile([128, 2, 512], F32R, tag="h3T", name="h3T")
                diag4 = sb3.tile([128, 4, 128], F32R, tag="diag4", name="diag4")
                for s in range(nsub):
                    nc.gpsimd.tensor_scalar(out=diag4[0:sw, s, 0:sw],
                                            in0=ident4[0:sw, s, 0:sw],
                                            scalar1=rstd4[0:sw, s:s + 1],
                                            scalar2=None, op0=ALU.mult)
                mrp = pbig.tile([128, 512], F32R, tag="big", name="mrp")
                for s in range(nsub):
                    nc.tensor.matmul(mrp[0:1, s * sw:(s + 1) * sw],
                                     mr4[0:sw, s:s + 1],
                                     ident_r[0:sw, 0:sw], is_transpose=True,
                                     skip_group_check=(s > 0))
                mrow = sb3.tile([1, 512], F32R, tag="mrow", name="mrow")
                nc.vector.tensor_copy(mrow[0:1, 0:nsub * sw], mrp[0:1, 0:nsub * sw])
                for s in range(nsub):
                    pt = ptr.tile([128, 256], F32R, tag="tr", name="pt")
                    for dc in range(2):
                        nc.tensor.matmul(pt[:, dc * sw:(dc + 1) * sw],
                                         fxu4[0:sw, s, dc * 128:(dc + 1) * 128],
                                         diag4[0:sw, s, 0:sw], is_transpose=True,
                                         start=True, stop=False,
                                         skip_group_check=(dc == 1))
                    nc.tensor.matmul(pt[:, 0:2 * sw].bitcast(F32), nones[0:1, 0:128],
                                     _dup2(mrow[0:1, s * sw:(s + 1) * sw]),
                                     start=False, stop=True, skip_group_check=True)
                    if s % 2 == 0:
                        nc.vector.tensor_copy(h3T[:, :, s * 128:s * 128 + sw],
                                              pt[:, 0:2 * sw].rearrange("p (c w) -> p c w", c=2))
                    else:
                        nc.scalar.activation(h3T[:, :, s * 128:s * 128 + sw],
                                             pt[:, 0:2 * sw].rearrange("p (c w) -> p c w", c=2),
                                             AF.Copy)
                return h3T

            def back(C, h3T):
                T0, T, nsub, sw = chdim(C)
                foTps = pfo.tile([128, 2, 512], F32, tag="facc", name="foTps")
                uq = []
                def fomm(fs, uT):
                    for dc in range(2):
                        nc.tensor.matmul(foTps[:, dc, 0:T],
                                         m2[:, fs, dc * 128:(dc + 1) * 128],
                                         uT[:, 0:T], start=(fs == 0),
                                         stop=(fs == 7 and not flags["mb2"]),
                                         skip_group_check=(fs > 0))
                for fs in range(8):
                    ups = pbig.tile([128, 512], F32, tag="big", name="ups")
                    for dc in range(2):
                        nc.tensor.matmul(ups[:, 0:T], m1[:, dc, fs * 128:(fs + 1) * 128],
                                         h3T[:, dc, 0:T], start=(dc == 0), stop=(dc == 1))
                    uT = sb3.tile([128, 512], F32R, tag="uT", name="uT")
                    nc.scalar.activation(uT[:, 0:T], ups[:, 0:T], AF.Gelu,
                                         bias=ib2[:, fs:fs + 1])
                    uq.append((fs, uT))
                    if len(uq) >= 3:
                        fomm(*uq.pop(0))
                while uq:
                    fomm(*uq.pop(0))
                if flags["mb2"]:
                    for dc in range(2):
                        nc.tensor.matmul(foTps[:, dc, 0:T],
                                         mb2[:, dc * 128:(dc + 1) * 128],
                                         onesr_row[0:1, 0:T], start=False,
                                         stop=True, skip_group_check=True)
                foT = sb.tile([128, 2, 512], F32, tag="foT", name="foT")
                nc.scalar.activation(foT[:, 0, 0:T], foTps[:, 0, 0:T], AF.Copy)
                nc.vector.tensor_copy(foT[:, 1, 0:T], foTps[:, 1, 0:T])
                nc.sync.dma_start(out=fxo_d[:, :, T0:T0 + T].rearrange("c p t -> p c t"),
                                  in_=foT[:, :, 0:T])

            st = front_a(0)
            h3T_c = front_b(st)
            for C in range(NCH2):
                bk = h3T_c
                nxt = front_a(C + 1) if C + 1 < NCH2 else None
                back(C, bk)
                if nxt is not None:
                    h3T_c = front_b(nxt)

    nc.finalize()
    return nc


_NC_CACHE = {}


def _get_nc(which, flags):
    key = (which, tuple(sorted(flags.items())))
    if key not in _NC_CACHE:
        _NC_CACHE[key] = build_launch1(flags) if which == 1 else build_launch2(flags)
    return _NC_CACHE[key]


def kernel(**inputs):
    inp = {k: np.ascontiguousarray(np.asarray(v)) for k, v in inputs.items()}
    x, fx = inp["x"], inp["fx"]
    f64 = lambda k: inp[k].astype(np.float64)

    # ---- host-side weight folding (LN gains into following weights) ----
    g1, b1 = f64("ln1_g"), f64("ln1_b")
    g2, b2 = f64("ln2_g"), f64("ln2_b")
    g3, b3 = f64("ln3_g"), f64("ln3_b")
    Wq, Wk, Wv = f64("Wq"), f64("Wk"), f64("Wv")
    wqkv = np.concatenate([g1[:, None] * Wq, g1[:, None] * Wk, g1[:, None] * Wv],
                          axis=1).astype(np.float32)
    bqkv = np.concatenate([b1 @ Wq, b1 @ Wk, b1 @ Wv]).astype(np.float32)[None, :]
    w1 = (g2[:, None] * f64("mlp_W1")).astype(np.float32)
    ib1 = (b2 @ f64("mlp_W1") + f64("mlp_b1")).astype(np.float32)
    m1 = (g3[:, None] * f64("mlp2_W1")).astype(np.float32)
    ib2 = (b3 @ f64("mlp2_W1") + f64("mlp2_b1")).astype(np.float32)
    cmask = np.zeros((D, D), np.float32)
    for h in range(H):
        cmask[h * DH:(h + 1) * DH, h * DH:(h + 1) * DH] = DH ** -0.5

    flags1 = {"bqkv": bool(np.any(bqkv)), "bo": bool(np.any(inp["bo"])),
              "b2": bool(np.any(inp["mlp_b2"])),
              "pb2": bool(np.any(inp["proj_b2"]))}
    flags1["anybias"] = (any(flags1.values()) or bool(np.any(ib1))
                         or bool(np.any(inp["proj_b1"])))
    xp = np.zeros((B, NP_, D), np.float32); xp[:, :N] = x
    fxp = np.zeros((B, NP_, D), np.float32); fxp[:, :N] = fx

    flags2 = {"mb2": bool(np.any(inp["mlp2_b2"]))}

    common1 = {
        "wqkv": wqkv, "wo": inp["Wo"], "w1": w1, "w2": inp["mlp_W2"],
        "p1": inp["proj_W1"], "p2": inp["proj_W2"], "cmask": cmask,
        "ib1": ib1, "ip1": inp["proj_b1"], "ipb2": inp["proj_b2"],
    }
    if flags1["bqkv"]:
        common1["bqkv"] = bqkv
    if flags1["bo"]:
        common1["bo"] = inp["bo"][None, :].astype(np.float32)
    if flags1["b2"]:
        common1["b2"] = inp["mlp_b2"][None, :].astype(np.float32)

    nc1 = _get_nc(1, flags1)
    in_maps1 = [dict(common1, x=xp[b], fx=fxp[b]) for b in range(B)]
    res1 = run_bass_kernel_spmd(nc1, in_maps1, CORES).results
    res1 = [{k: np.asarray(v) for k, v in r.items()} for r in res1]

    # ---- host boundary: cov all-reduce + Cholesky + M fold ----
    cov = sum(r["cov"].astype(np.float64) for r in res1) / (B * N)
    L = np.linalg.cholesky(cov)
    Linv = np.linalg.inv(L)
    sp_mu = np.log1p(np.exp(inp["mu"].astype(np.float64)))
    M = Linv.T @ (sp_mu[:, None] * Linv)

    common2 = {"m1": m1, "m2": inp["mlp2_W2"], "ib2": ib2}
    if flags2["mb2"]:
        common2["mb2"] = inp["mlp2_b2"][None, :].astype(np.float32)
    nc2 = _get_nc(2, flags2)
    in_maps2 = [dict(common2, xt=res1[b]["xt"],
                     c2pp=(M @ res1[b]["c2p"].astype(np.float64)).astype(np.float32))
                for b in range(B)]
    res2 = run_bass_kernel_spmd(nc2, in_maps2, CORES).results
    res2 = [{k: np.asarray(v) for k, v in r.items()} for r in res2]

    # outputs are feature-major [2, 128, NP_]; de-transpose on host
    x_out = np.stack([res1[b]["x2o"].reshape(D, NP_).T[:N] for b in range(B)]
                     ).astype(np.float32)
    fx_out = np.stack([res2[b]["fxo"].reshape(D, NP_).T[:N] for b in range(B)]
                      ).astype(np.float32)
    return x_out, fx_out
